# revision 38
# baseline (speedup 1.0000x reference)
"""Trainium2 Bass kernel for the DifferentiableTree module.

Math (per sample s, heap-ordered perfect binary tree, depth 10):
  gate[s, i] = 0.5*(1 + erf((thr[i] - mean[s, f(i)]) / (std[s, f(i)]*sqrt(2))))
  w = path products over levels; pred = (w @ leaf_values) row-normalized.

Kernel strategy (data-parallel over samples, 8 cores x 8192 samples), built
around a hybrid sample-major/node-major tree that shrinks the on-chip
transpose from 1024 to 128 values per sample:

  - One [128, 1024] fp16 R table yields every erf argument from a single
    lhsT = [1/std; mean/std] tensor: cols [0:128) are sample-major
    (dummy, root, levels 1-6) via matmul(lhsT=ab, rhs=R0); cols [128:1024)
    are seven NODE-major blocks (level-7/8/9 gates, suffix-indexed per
    level-7 prefix p) via swapped matmuls (lhsT=R-block, rhs=ab).  All 8
    products land in one [128, 1024] fp32 PSUM tile -> a single Erf per
    chunk (ScalarE runs nothing but Erf; a warm-up erf at t=0 hides the
    activation-table load).
  - Tree levels 1-6 run sample-major in place (weights stay on sample
    partitions); level-6 outputs land in a contiguous C tile, so ONE
    DMA-xbar transpose per supertile moves just w7 (128/sample) to
    leaf-major.  Levels 7-9 then combine node-major: gates are already
    transposed, ops stay partition-aligned via (prefix, suffix) indexing.
  - The last level folds into the leaf matmul: pred = sum_p w9_p*(L_l+L_r)/2
    + (w9_p*e9_p)*(L_l-L_r)/2 with raw erf output e9 -- no level-9 subtract,
    no affine for level-9 gates.
  - Chunks split between DVE and the otherwise-idle Pool engine (the Pool
    ISA only allows tensor_tensor / immediate tensor_scalar, so both
    classes run the same affined mul/sub algebra on their own chunks).
    The mean/std front multiply also rides on Pool.
  - Row normalization: per-chunk rowsums come from side 1-column matmuls
    (sharing Ldweights with the main ones); a quad-batched DVE reciprocal
    + stride-0-broadcast TT multiply applies the scale (ALU divide and
    per-partition-scalar ops from PSUM are not ISA-legal on DVE/Pool).
  - Depth-3 software pipeline: iteration i runs tree(i) on DVE/Pool while
    PE interleaves back-matmuls of supertile i-1 (two half-supertile
    waves, quad-sized single-bank PSUM pred tiles) into the erf-paced
    front stream of supertile i+1; normalize waves slot between
    node-major ops so no engine bubbles on cross-engine deps.  Tapered
    supertiles (8/16/16/12/6/4/2) shrink fill and drain.
"""

import numpy as np

N = 65536
N_CORES = 8
NS = N // N_CORES          # 8192 samples per core
F = 64
DEPTH = 10
NL = 2 ** DEPTH            # 1024 leaves
NCLS = 128
CHUNKS = NS // 128         # 64

_CACHE = {}


def _node_at(j, depth):
    h = 0
    for i in range(depth):
        h = 2 * h + 1 if (j >> i) & 1 else 2 * h + 2
    return h


def _build_tables(features, thresholds, leaf_values):
    """R [128, 1024] f16 arg table, LE [128, 8, 129] f16 leaf matrix."""
    features = np.asarray(features).astype(np.int64)
    thresholds = np.asarray(thresholds, dtype=np.float64)
    L = np.asarray(leaf_values, dtype=np.float64)

    R = np.zeros((128, 1024), dtype=np.float64)
    s = 1.0 / np.sqrt(2.0)

    def set_col(col, h):
        f = features[h]
        R[f, col] = thresholds[h] * s
        R[64 + f, col] = -s

    set_col(1, 0)
    for l in range(1, 7):
        for j in range(2 ** l):
            set_col(2 ** l + j, _node_at(j, l))
    n7 = [_node_at(p, 7) for p in range(128)]
    n8, n9 = {}, {}
    for p in range(128):
        for a in range(2):
            n8[(p, a)] = 2 * n7[p] + (1 if a == 0 else 2)
            for b in range(2):
                n9[(p, a, b)] = 2 * n8[(p, a)] + (1 if b == 0 else 2)
    for p in range(128):
        set_col(128 + p, n7[p])
        set_col(256 + p, n8[(p, 0)])
        set_col(384 + p, n8[(p, 1)])
        for a in range(2):
            for b in range(2):
                set_col(512 + (2 * a + b) * 128 + p, n9[(p, a, b)])

    # reference leaf row for path (p bits 0..6, a, b, c); bit=1 -> g-branch
    # -> reference digit 0
    def row_ref(p, a, b, c):
        r = 0
        for l in range(7):
            r += (1 - ((p >> l) & 1)) * 2 ** (9 - l)
        return r + a * 4 + b * 2 + c

    # in-place homes after the node-major combine: [B0..B6, w7T-slot]
    homes = [("w9", 0, 1), ("w9", 0, 0), ("w9", 1, 0),
             ("hi", 0, 0), ("hi", 0, 1), ("hi", 1, 0), ("hi", 1, 1),
             ("w9", 1, 1)]
    LE = np.zeros((128, 8, 129), dtype=np.float64)
    for p in range(128):
        for t, (kind, a, b) in enumerate(homes):
            l0 = L[row_ref(p, a, b, 0)]
            l1 = L[row_ref(p, a, b, 1)]
            row = (l0 + l1) / 2.0 if kind == "w9" else (l0 - l1) / 2.0
            LE[p, t, 0:128] = row
            LE[p, t, 128] = row.sum()
    return R.astype(np.float16), LE.astype(np.float16)


# supertile split: (base chunk, size, DVE-class count)
SPLITS = [(0, 8, 6), (8, 16, 12), (24, 16, 12), (40, 16, 12), (56, 8, 6)]


def _build_nc():
    import concourse.bacc as bacc
    import concourse.tile as tile
    from concourse import mybir

    f32 = mybir.dt.float32
    f16 = mybir.dt.float16
    AF = mybir.ActivationFunctionType
    OP = mybir.AluOpType

    nc = bacc.Bacc("TRN2", target_bir_lowering=False, debug=False,
                   num_devices=N_CORES)
    X_d = nc.dram_tensor("X", [NS, 128], f16, kind="ExternalInput")
    R_d = nc.dram_tensor("R", [128, NL], f16, kind="ExternalInput")
    LE_d = nc.dram_tensor("LE", [128, 8, 129], f16, kind="ExternalInput")
    O_d = nc.dram_tensor("OUT", [NS, 128], f32, kind="ExternalOutput")

    with tile.TileContext(nc) as tc:
        with (
            tc.tile_pool(name="consts", bufs=1) as consts,
            tc.tile_pool(name="xp", bufs=6) as xp,
            tc.tile_pool(name="abp", bufs=5) as abp,
            tc.tile_pool(name="ep", bufs=3) as ep,
            tc.tile_pool(name="cp", bufs=3) as cp,
            tc.tile_pool(name="wtp", bufs=4) as wtp,
            tc.tile_pool(name="op", bufs=8) as op,
            tc.tile_pool(name="rcpp", bufs=4) as rcpp,
            tc.tile_pool(name="argps", bufs=2, space="PSUM") as argps,
            tc.tile_pool(name="predps", bufs=2, space="PSUM") as predps,
            tc.tile_pool(name="rowps", bufs=2, space="PSUM") as rowps,
        ):
            r_sb = consts.tile([128, NL], f16)
            le_sb = consts.tile([128, 8, 129], f16)
            warm = consts.tile([1, 1], f16)
            nc.vector.memset(warm, 0.0)
            nc.scalar.activation(warm, warm, AF.Erf)  # act-table warm-up

            def front_ab(st, dve_bmul=False, gmax=8):
                """X loads + 1/std (DVE) + mean/std (Pool) for supertile."""
                base, sz, _ = st
                abs_ = []
                sizes = []
                left = sz
                while left > 0:
                    g = min(gmax, left)
                    sizes.append(g)
                    left -= g
                for q, gsz in enumerate(sizes):
                    xt = xp.tile([128, gsz, 128], f16, tag="xt", name="xt")
                    ab = abp.tile([128, gsz, 128], f16, tag="ab", name="ab")
                    g0 = (base + sum(sizes[:q])) * 128
                    nc.sync.dma_start_transpose(
                        xt, X_d[g0:g0 + gsz * 128, :])
                    with nc.allow_low_precision(reason="fp16 1/std"):
                        nc.vector.reciprocal(ab[0:64, :, :], xt[64:128, :, :])
                    beng = nc.vector if dve_bmul else nc.gpsimd
                    beng.tensor_mul(ab[64:128, :, :], xt[0:64, :, :],
                                    ab[0:64, :, :])
                    abs_.append((ab, gsz))
                return abs_

            def abr_of(abs_, c):
                """chunk c's ab row within the group list."""
                for ab, gsz in abs_:
                    if c < gsz:
                        return ab[:, c, :]
                    c -= gsz
                raise IndexError

            def front_mm(E, abs_, c0, c1):
                """arg matmuls + erf for chunks [c0, c1) of the supertile."""
                for c in range(c0, c1):
                    arg = argps.tile([128, NL], f32, tag="arg", name="arg")
                    abr = abr_of(abs_, c)
                    nc.tensor.matmul(arg[:, 0:128], abr, r_sb[:, 0:128])
                    for t in range(7):
                        nc.tensor.matmul(
                            arg[:, 128 + 128 * t:256 + 128 * t],
                            r_sb[:, 128 + 128 * t:256 + 128 * t], abr)
                    nc.scalar.activation(E[:, c, :], arg, AF.Erf)

            def tree_phase1(st, E, C):
                """affine/root + sample-major levels 1-6 (into C)."""
                _, sz, nd = st
                nc.vector.tensor_scalar(E[:, :, 0:512], E[:, :, 0:512],
                                        0.5, 0.5, OP.mult, OP.add)
                nc.vector.tensor_scalar(E[:, :, 0:1], E[:, :, 1:2],
                                        -1.0, 1.0, OP.mult, OP.add)
                for eng, cs in ((nc.vector, slice(0, nd)),
                                (nc.gpsimd, slice(nd, sz))):
                    ev = E[:, cs, :]
                    cv = C[:, cs, :]
                    for l in range(1, 7):
                        d = 2 ** l
                        lo, g = ev[:, :, 0:d], ev[:, :, d:2 * d]
                        if l < 6:
                            eng.tensor_mul(g, lo, g)
                            eng.tensor_sub(lo, lo, g)
                        else:
                            hi = cv[:, :, 64:128]
                            eng.tensor_mul(hi, lo, g)
                            eng.tensor_sub(cv[:, :, 0:64], lo, hi)

            def node_major_ops(st, E, W, eng_sel):
                """levels 7-9 leaf-major; in-place homes [B0..B6, w7T]."""
                _, sz, nd = st
                ops = []
                engs = {"dve": (nc.vector, slice(0, nd)),
                        "pool": (nc.gpsimd, slice(nd, sz))}
                eng, cs = engs[eng_sel]
                if cs.start >= cs.stop:
                    return ops
                B = [E[:, cs, 128 + 128 * t:256 + 128 * t] for t in range(7)]
                Wc = W[:, cs, :]
                ops.append(lambda: eng.tensor_mul(B[0], Wc, B[0]))   # w8_0
                ops.append(lambda: eng.tensor_sub(Wc, Wc, B[0]))     # w8_1
                ops.append(lambda: eng.tensor_mul(B[1], B[0], B[1]))  # w9_00
                ops.append(lambda: eng.tensor_sub(B[0], B[0], B[1]))  # w9_01
                ops.append(lambda: eng.tensor_mul(B[2], Wc, B[2]))   # w9_10
                ops.append(lambda: eng.tensor_sub(Wc, Wc, B[2]))     # w9_11
                ops.append(lambda: eng.tensor_mul(B[3], B[1], B[3]))  # HI_00
                ops.append(lambda: eng.tensor_mul(B[4], B[0], B[4]))  # HI_01
                ops.append(lambda: eng.tensor_mul(B[5], B[2], B[5]))  # HI_10
                ops.append(lambda: eng.tensor_mul(B[6], Wc, B[6]))   # HI_11
                return ops

            def half_quads(st, h):
                nq = (st[1] + 3) // 4
                nq0 = (nq + 1) // 2
                return range(0, nq0) if h == 0 else range(nq0, nq)

            def back_mm_half(bk, h):
                """PE matmuls for half h of the supertile's chunk-quads."""
                E, W = bk["E"], bk["W"]
                sz = bk["st"][1]
                quads = [q for q in half_quads(bk["st"], h) if 4 * q < sz]
                if not len(quads):
                    return

                def lhs(c, t):
                    if t < 7:
                        return E[:, c, 128 + 128 * t:256 + 128 * t]
                    return W[:, c, :]

                rows = rowps.tile([128, 4 * len(quads), 1], f32, tag="rows",
                                  name="rows")
                bk["rows"][h] = rows
                for cq in quads:
                    qn = min(4, sz - 4 * cq)
                    pred = predps.tile([128, 4, 128], f32, tag="pred",
                                       name="pred")
                    bk["preds"][cq] = pred
                    for k2 in range(qn):
                        c = cq * 4 + k2
                        for t in range(8):
                            nc.tensor.matmul(pred[:, k2, :], lhs(c, t),
                                             le_sb[:, t, 0:128],
                                             start=(t == 0), stop=(t == 7))
                        for t in range(8):
                            nc.tensor.matmul(
                                rows[:, (cq - quads[0]) * 4 + k2, :],
                                lhs(c, t), le_sb[:, t, 128:129],
                                start=(t == 0), stop=(t == 7))

            def back_fin_half(bk, h):
                """normalize (DVE, some quads on Act) + output DMA."""
                base, sz, _ = bk["st"]
                quads = [q for q in half_quads(bk["st"], h) if 4 * q < sz]
                if not len(quads):
                    return
                rows = bk["rows"][h]
                rcp = rcpp.tile([128, 4 * len(quads), 1], f32, tag="rcp",
                                name="rcp")
                nc.vector.reciprocal(rcp, rows)
                for cq in quads:
                    qn = min(4, sz - 4 * cq)
                    pred = bk["preds"][cq]
                    o = op.tile([128, 4, 128], f32, tag="o", name="o")
                    r0 = (cq - quads[0]) * 4
                    if sz == 16 and cq % 4 == 1:
                        for k2 in range(qn):
                            nc.scalar.activation(
                                o[:, k2, :], pred[:, k2, :], AF.Copy,
                                scale=rcp[:, r0 + k2, :])
                    else:
                        nc.vector.tensor_mul(
                            o[:, 0:qn, :], pred[:, 0:qn, :],
                            rcp[:, r0:r0 + qn, :].broadcast_to(
                                (128, qn, 128)))
                    g0 = (base + cq * 4) * 128
                    nc.sync.dma_start(
                        out=O_d[g0:g0 + qn * 128, :].rearrange(
                            "(k p) f -> p k f", p=128),
                        in_=o[:, 0:qn, :])

            def make_back(st, E, W):
                return {"st": st, "E": E, "W": W,
                        "rows": [None, None],
                        "preds": [None] * ((st[1] + 3) // 4)}

            # depth-3 pipeline: iteration i runs tree(i) on DVE/Pool,
            # front(i+1) erf-paced on PE/Act with back(i-1) halves
            # interleaved, and normalize(i-1) waves inside node-major(i).
            E = {}
            E[0] = ep.tile([128, SPLITS[0][1], NL], f16, tag="E", name="E")
            nc.sync.dma_start(out=r_sb, in_=R_d[:, :])
            nc.sync.dma_start(out=le_sb, in_=LE_d[:, :, :])
            abs0 = front_ab(SPLITS[0], dve_bmul=True, gmax=2)
            front_mm(E[0], abs0, 0, SPLITS[0][1])
            bk_prev = None
            for i, st in enumerate(SPLITS):
                base, sz, nd = st
                nxt = SPLITS[i + 1] if i + 1 < len(SPLITS) else None
                C = cp.tile([128, sz, 128], f16, tag="C", name="C")
                W = wtp.tile([128, sz, 128], f16, tag="W", name="W")
                abs_n = front_ab(nxt) if nxt else None
                tree_phase1(st, E[i], C)
                if nxt:
                    E[i + 1] = ep.tile([128, nxt[1], NL], f16, tag="E",
                                       name="E")
                    ngroups = [(g, min(g + 4, nxt[1]))
                               for g in range(0, nxt[1], 4)]
                    emitted = 0
                    for gi, (g0, g1) in enumerate(ngroups):
                        front_mm(E[i + 1], abs_n, g0, g1)
                        if bk_prev and gi % 2 == 1 and emitted < 2:
                            back_mm_half(bk_prev, emitted)
                            emitted += 1
                    if bk_prev:
                        for h in range(emitted, 2):
                            back_mm_half(bk_prev, h)
                else:
                    if bk_prev:
                        back_mm_half(bk_prev, 0)
                        back_mm_half(bk_prev, 1)
                nc.sync.dma_start_transpose(W[:, 0:nd, :], C[:, 0:nd, :])
                if nd < sz:
                    nc.sync.dma_start_transpose(W[:, nd:sz, :],
                                                C[:, nd:sz, :])
                # node-major with normalize waves of (i-1) slotted between
                dve_ops = node_major_ops(st, E[i], W, "dve")
                pool_ops = node_major_ops(st, E[i], W, "pool")
                for f in pool_ops:
                    f()
                for f in dve_ops[:5]:
                    f()
                if bk_prev:
                    back_fin_half(bk_prev, 0)
                for f in dve_ops[5:]:
                    f()
                if bk_prev:
                    back_fin_half(bk_prev, 1)
                bk_prev = make_back(st, E[i], W)
            back_mm_half(bk_prev, 0)
            back_fin_half(bk_prev, 0)
            back_mm_half(bk_prev, 1)
            back_fin_half(bk_prev, 1)

    nc.compile()
    return nc


def kernel(X, features, thresholds, leaf_values, trace=False):
    from concourse.bass_utils import run_bass_kernel_spmd

    X = np.ascontiguousarray(
        np.asarray(X, dtype=np.float32).astype(np.float16))
    R, LE = _build_tables(features, thresholds, leaf_values)

    if "nc" not in _CACHE:
        _CACHE["nc"] = _build_nc()
    nc = _CACHE["nc"]

    in_maps = [
        {"X": X[c * NS:(c + 1) * NS], "R": R, "LE": LE}
        for c in range(N_CORES)
    ]
    res = run_bass_kernel_spmd(nc, in_maps, core_ids=list(range(N_CORES)),
                               trace=trace)
    out = np.concatenate([res.results[c]["OUT"] for c in range(N_CORES)],
                         axis=0)
    _CACHE["last_results"] = res
    return out
